# revision 2
# baseline (speedup 1.0000x reference)
"""GQA ragged-cache attention kernel for 8 Trainium2 NeuronCores.

Sharding: 16 (batch, kv-head) pairs across 8 cores, 2 pairs/core (tensor
parallel over KVH heads x data parallel over B, per the sharding hint).

Trick: the host rotates each pair's KV cache by seq_len so the ragged
causal mask becomes: (a) a static causal mask over the first 1024 rotated
slots (the new tokens, applied with gpsimd affine_select), plus (b) a
per-row -1e30 exp bias over the last 1024 slots (old cache; valid rows
depend only on the row index, never the query) supplied as data. All
control flow is static; scatter of new K/V into the cache becomes a pure
copy in rotated space that the host un-rotates afterwards.
"""

import numpy as np
from contextlib import ExitStack

B, QL, L = 4, 1024, 2048
H, KVH, D = 16, 4, 128
G = H // KVH
NP_CORES = 8
PAIRS_PER_CORE = 2
NT = L // 128  # 16 l'-tiles
SCALE = 1.0 / float(np.sqrt(D))

_CACHE = {}


def _build_program():
    import concourse.bass as bass
    import concourse.tile as tile
    from concourse import bacc, mybir

    F32, F32R = mybir.dt.float32, mybir.dt.float32r
    nc = bacc.Bacc("TRN2", target_bir_lowering=False, debug=False,
                   num_devices=NP_CORES)

    qT_d = nc.dram_tensor("qT", [2, G, 128, QL], F32R, kind="ExternalInput").ap()
    KT_d = nc.dram_tensor("KT", [2, 128, L], F32R, kind="ExternalInput").ap()
    Kn_d = nc.dram_tensor("Kn", [2, L, D], F32, kind="ExternalInput").ap()
    Vn_d = nc.dram_tensor("Vn", [2, L, D], F32, kind="ExternalInput").ap()
    bias_d = nc.dram_tensor("bias", [2, NT, 128], F32, kind="ExternalInput").ap()
    ones_d = nc.dram_tensor("ones", [128, 1], F32R, kind="ExternalInput").ap()
    outT_d = nc.dram_tensor("outT", [2, G, 128, QL], F32, kind="ExternalOutput").ap()
    aK_d = nc.dram_tensor("aK", [2, L, D], F32, kind="ExternalOutput").ap()
    aV_d = nc.dram_tensor("aV", [2, L, D], F32, kind="ExternalOutput").ap()

    EXP = mybir.ActivationFunctionType.Exp
    GE = mybir.AluOpType.is_ge
    MUL = mybir.AluOpType.mult

    with tile.TileContext(nc) as tc:
        with ExitStack() as ctx:
            sbc = ctx.enter_context(tc.tile_pool(name="sbc", bufs=2))
            sbq = ctx.enter_context(tc.tile_pool(name="sbq", bufs=2))
            sbe = ctx.enter_context(tc.tile_pool(name="sbe", bufs=3))
            sbo = ctx.enter_context(tc.tile_pool(name="sbo", bufs=3))
            sbs = ctx.enter_context(tc.tile_pool(name="sbs", bufs=2))
            psS = ctx.enter_context(tc.tile_pool(name="psS", bufs=2, space="PSUM"))
            psP = ctx.enter_context(tc.tile_pool(name="psP", bufs=1, space="PSUM"))
            psD = ctx.enter_context(tc.tile_pool(name="psD", bufs=1, space="PSUM"))

            ones_t = sbc.tile([128, 1], F32R, name="ones_t", tag="ones")
            nc.sync.dma_start(ones_t[:], ones_d[:])

            for p in range(PAIRS_PER_CORE):
                KT_t = sbc.tile([128, L], F32R, name=f"KT{p}", tag="KT")
                nc.sync.dma_start(KT_t[:], KT_d[p])
                V_t = sbc.tile([128, NT, D], F32R, name=f"V{p}", tag="V")
                nc.gpsimd.dma_start(
                    V_t[:], Vn_d[p].rearrange("(t p) d -> p t d", p=128))
                b_t = sbc.tile([128, NT], F32, name=f"b{p}", tag="bias")
                nc.sync.dma_start(b_t[:], bias_d[p].rearrange("t l -> l t"))
                q_ts = []
                for g in range(G):
                    qg = sbq.tile([128, QL], F32R, name=f"q{p}{g}", tag=f"q{g}")
                    nc.sync.dma_start(qg[:], qT_d[p, g])
                    q_ts.append(qg)

                # cache update: copy rotated K/V through SBUF (host un-rotates)
                for nm, src, dst in (("k", Kn_d, aK_d), ("v", Vn_d, aV_d)):
                    st = sbs.tile([128, NT, D], F32, name=f"cp{nm}{p}", tag="cp")
                    nc.sync.dma_start(
                        st[:], src[p].rearrange("(t p) d -> p t d", p=128))
                    nc.sync.dma_start(
                        dst[p].rearrange("(t p) d -> p t d", p=128), st[:])

                for q0 in range(2):
                    # live l'-tiles: q0=0 skips causally-dead t in [4, 8)
                    ts = [t for t in range(NT) if not (q0 == 0 and 4 <= t < 8)]
                    for gp in range(2):
                        PV, DEN = [], []
                        for i in range(2):
                            PV.append(psP.tile([128, 512], F32,
                                               name=f"pv{p}{q0}{gp}{i}", tag=f"pv{i}"))
                            DEN.append(psD.tile([128, 512], F32,
                                                name=f"dn{p}{q0}{gp}{i}", tag=f"dn{i}"))
                        for t in ts:
                            S_t = psS.tile([128, 1024], F32,
                                           name=f"s{p}{q0}{gp}{t}", tag="S")
                            for i in range(2):
                                nc.tensor.matmul(
                                    S_t[:, i * 512:(i + 1) * 512],
                                    KT_t[:, t * 128:(t + 1) * 128],
                                    q_ts[gp * 2 + i][:, q0 * 512:(q0 + 1) * 512],
                                    start=True, stop=True)
                            E_t = sbe.tile([128, 1024], F32R,
                                           name=f"e{p}{q0}{gp}{t}", tag="E")
                            nc.scalar.activation(E_t[:], S_t[:], EXP,
                                                 bias=b_t[:, t:t + 1], scale=SCALE)
                            if t < 8 and t * 128 + 127 > q0 * 512:
                                # causal band: keep iff 128t+li <= 512q0+qi
                                nc.gpsimd.affine_select(
                                    E_t[:].rearrange("p (g q) -> p g q", g=2),
                                    E_t[:].rearrange("p (g q) -> p g q", g=2),
                                    pattern=[[0, 2], [1, 512]],
                                    compare_op=GE, fill=0.0,
                                    base=q0 * 512 - t * 128,
                                    channel_multiplier=-1)
                            for i in range(2):
                                sl_e = E_t[:, i * 512:(i + 1) * 512]
                                nc.tensor.matmul(PV[i][:], V_t[:, t, :], sl_e,
                                                 start=(t == 0), stop=(t == NT - 1))
                                nc.tensor.matmul(DEN[i][0:1, :], ones_t[:], sl_e,
                                                 start=(t == 0), stop=(t == NT - 1))
                        for i in range(2):
                            rec = sbo.tile([1, 512], F32,
                                           name=f"rc{p}{q0}{gp}{i}", tag="rec")
                            nc.vector.reciprocal(rec[:], DEN[i][0:1, :])
                            dnb = sbo.tile([128, 512], F32,
                                           name=f"db{p}{q0}{gp}{i}", tag="dnb")
                            nc.gpsimd.partition_broadcast(dnb[:], rec[:])
                            o_t = sbo.tile([128, 512], F32,
                                           name=f"o{p}{q0}{gp}{i}", tag="o")
                            nc.vector.tensor_tensor(out=o_t[:], in0=PV[i][:],
                                                    in1=dnb[:], op=MUL)
                            nc.sync.dma_start(
                                outT_d[p, gp * 2 + i, :, q0 * 512:(q0 + 1) * 512],
                                o_t[:])
    nc.compile()
    return nc


def _get_nc():
    if "nc" not in _CACHE:
        _CACHE["nc"] = _build_program()
    return _CACHE["nc"]


def _prep_core_inputs(c, q, k, v, cache_k, cache_v, seq_lens):
    ins = {"qT": np.empty((2, G, 128, QL), np.float32),
           "KT": np.empty((2, 128, L), np.float32),
           "Kn": np.empty((2, L, D), np.float32),
           "Vn": np.empty((2, L, D), np.float32),
           "bias": np.zeros((2, NT, 128), np.float32),
           "ones": np.ones((128, 1), np.float32)}
    for j in range(PAIRS_PER_CORE):
        pg = c * PAIRS_PER_CORE + j
        b, kv = pg // KVH, pg % KVH
        sl = int(seq_lens[b])
        idx_old = (np.arange(QL, L) + sl) % L
        ins["Kn"][j] = np.concatenate(
            [k[b, :, kv * D:(kv + 1) * D], cache_k[b, kv][idx_old]], 0)
        ins["Vn"][j] = np.concatenate(
            [v[b, :, kv * D:(kv + 1) * D], cache_v[b, kv][idx_old]], 0)
        ins["KT"][j] = ins["Kn"][j].T
        for g in range(G):
            h = kv * G + g
            ins["qT"][j, g] = q[b, :, h * D:(h + 1) * D].T
        i_all = np.arange(L).reshape(NT, 128)
        dead = (i_all >= QL) & (i_all < L - sl)
        ins["bias"][j][dead] = -1e30
    return {n: np.ascontiguousarray(a) for n, a in ins.items()}


def kernel(q, k, v, cache_k, cache_v, seq_lens):
    from concourse import bass_utils
    q, k, v = np.asarray(q), np.asarray(k), np.asarray(v)
    cache_k, cache_v = np.asarray(cache_k), np.asarray(cache_v)
    seq_lens = np.asarray(seq_lens)
    nc = _get_nc()
    in_maps = [_prep_core_inputs(c, q, k, v, cache_k, cache_v, seq_lens)
               for c in range(NP_CORES)]
    res = bass_utils.run_bass_kernel_spmd(nc, in_maps,
                                          core_ids=list(range(NP_CORES)))
    out = np.empty((B, QL, H * D), np.float32)
    new_ck = np.empty((B, KVH, L, D), np.float32)
    new_cv = np.empty((B, KVH, L, D), np.float32)
    for c in range(NP_CORES):
        r = res.results[c]
        for j in range(PAIRS_PER_CORE):
            pg = c * PAIRS_PER_CORE + j
            b, kv = pg // KVH, pg % KVH
            sl = int(seq_lens[b])
            unrot = (np.arange(L) - sl) % L
            new_ck[b, kv] = r["aK"][j][unrot]
            new_cv[b, kv] = r["aV"][j][unrot]
            for g in range(G):
                h = kv * G + g
                out[b, :, h * D:(h + 1) * D] = r["outT"][j, g].T
    return out, new_ck, new_cv
